# revision 38
# baseline (speedup 1.0000x reference)
"""Trainium2 Bass kernel for the 4-layer LIF spiking network (EventDrivenSparseNetwork).

Strategy:
  - Data-parallel over batch: B=32 sharded 4-per-core across 8 NeuronCores,
    weights replicated. No cross-core communication.
  - All GEMMs run at full PE rate (1 cycle/row) via fp16 term-splitting;
    fp32 matmuls (4 cycles/row) are eliminated entirely:
      * layers 1-3: inputs are exact 0/1 spikes (stored fp16), weights are
        split w*2^12 ~= h0 + 2^-11*h1 (fp16 each) -> 2 full-rate matmuls
        into 2 PSUM banks; products are exact, so effective weight
        precision is ~22 bits (+ optional 3rd term -> 33 bits).
      * layer 0: gaussian input x is also split x ~= x0 + 2^-11*x1 (fp16),
        cur = h0x0 + 2^-11(h0x1 + h1x0) [+ 2^-22 h1x1] -> 3 (or 4)
        full-rate matmuls; dropped terms are O(2^-22), i.e. fp32-noise.
  - Weight terms are pre-scaled by 2^12 so h1 (and tiny h0) stay out of
    the fp16 subnormal range; the LIF threshold becomes 2^12 (power-of-2
    scaling is exact in fp32, so scan numerics are unchanged).
  - The LIF scan runs on VectorE: v = alpha*v + cur_t; s = (v >= TH);
    v = (v < TH)*v.  T is processed in 2 chunks so the scan of chunk c
    overlaps the GEMM of chunk c+1 / the next layer's first chunk.
  - All transposes / masking / splitting / sharding happen host-side in
    numpy; the device sees contiguous DMA-friendly layouts.
"""

import os
import sys

sys.path.insert(0, "/opt/trn_rl_repo")

import numpy as np

B, T, N = 32, 100, 2048
NL = 4
NCORES = 8
BL = B // NCORES          # 4 samples per core
NO = N // 128             # 16 output-neuron chunks
KO = N // 128             # 16 contraction chunks
F = T * BL                # 400, f = t*BL + b
ALPHA = float(np.float32(np.exp(np.float32(-1.0 / 20.0))))
TSPLIT = tuple(int(t) for t in os.environ.get("LIF_TSPLIT", "50,50").split(","))
TSPLIT3 = tuple(int(t)
                for t in os.environ.get("LIF_TSPLIT3", "50,30,20").split(","))
NT = int(os.environ.get("LIF_NTERMS", "2"))      # weight fp16 terms (2 or 3)
L0MM = int(os.environ.get("LIF_L0MM", "3"))      # layer-0 matmuls (3 or 4)
RES = int(os.environ.get("LIF_RES", "9"))        # mo tiles resident per layer
WSRING = int(os.environ.get("LIF_WSRING", "4"))  # streamed-weight ring bufs
GRP = int(os.environ.get("LIF_GRP", "2"))        # mo interleave group (L1-3)
SCANP = int(os.environ.get("LIF_SCANP", "1"))    # spike op on Pool engine
WSCALE = float(2.0 ** 12)
THRESH = float(2.0 ** 12)
S1 = float(2.0 ** -11)


def build(reps: int = 1, tsplit=TSPLIT, tsplit3=TSPLIT3, nterms=NT,
          l0mm=L0MM):
    """Build (and bacc-compile) the SPMD kernel. Returns the Bass object."""
    import contextlib
    from concourse import mybir, bacc
    import concourse.tile as tile

    assert sum(tsplit) == T and sum(tsplit3) == T
    layer_ts = [tsplit] * (NL - 1) + [tsplit3]
    fmax = max(t * BL for ts_ in layer_ts for t in ts_)
    f16 = mybir.dt.float16

    nc = bacc.Bacc("TRN2", target_bir_lowering=False, debug=False,
                   num_devices=NCORES)
    wh_d = nc.dram_tensor("wh", [NL, NO, 128, nterms, KO, 128], f16,
                          kind="ExternalInput").ap()
    x_d = nc.dram_tensor("x", [2, 128, KO, F], f16,
                         kind="ExternalInput").ap()
    out_d = nc.dram_tensor("out", [128, NO, F], mybir.dt.float32,
                           kind="ExternalOutput").ap()

    with tile.TileContext(nc) as tctx:
        with contextlib.ExitStack() as stack:
            actsp = stack.enter_context(tctx.tile_pool(name="acts", bufs=3))
            xp = stack.enter_context(tctx.tile_pool(name="xp", bufs=1))
            wp = stack.enter_context(tctx.tile_pool(name="wp", bufs=6))
            curp = stack.enter_context(tctx.tile_pool(name="curp", bufs=2))
            vp = stack.enter_context(tctx.tile_pool(name="vp", bufs=2))
            tp = stack.enter_context(tctx.tile_pool(name="tp", bufs=2))
            pp = stack.enter_context(tctx.tile_pool(name="pp", bufs=4,
                                                    space="PSUM"))

            PBUFS = (4, 4)

            def body(_iv=None):
                # x loads ride the ACT HWDGE ring (separate from the SP ring
                # that streams weights) and are split per half so the first
                # layer-0 GEMM isn't stuck behind the full x transfer.
                xt = xp.tile([128, 2, KO, F], f16, tag="x01")
                for fh in range(2):
                    fs = slice(fh * (F // 2), (fh + 1) * (F // 2))
                    for h in range(2):
                        nc.scalar.dma_start(xt[:, h, :, fs], x_d[h][:, :, fs])

                def gemm_grp(l, mo0, wts, cur, cur_in, f0, fc,
                             dve_copy=False):
                    # h1 is pre-scaled host-side, so all weight terms
                    # accumulate into ONE PSUM bank per mo; layers 1-3 need
                    # no combine at all (ACT copies PSUM->SBUF). A group of
                    # mo is interleaved so consecutive matmuls hit different
                    # PSUM banks (same-bank back-to-back stalls on drain).
                    # Layer 0 keeps a second bank for the x low-half products
                    # (x1 carries a 2^11 scale to avoid fp16 subnormals).
                    g = len(wts)
                    mk = lambda nm: pp.tile(
                        [128, fc], mybir.dt.float32, tag="p0",
                        name=f"{nm}_{l}_{mo0}_{f0}", bufs=8)
                    ps = [mk(f"p{j}") for j in range(g)]
                    if l == 0:
                        qs = [mk(f"q{j}") for j in range(g)]
                        x0 = xt[:, 0]
                        x1 = xt[:, 1]
                        for ko in range(KO):
                            st, sp = ko == 0, ko == KO - 1
                            xs0 = x0[:, ko, f0:f0 + fc]
                            xs1 = x1[:, ko, f0:f0 + fc]
                            for p, wt in zip(ps, wts):
                                nc.tensor.matmul(p[:, :], wt[:, 0, ko, :],
                                                 xs0, start=st, stop=False)
                            for q, wt in zip(qs, wts):
                                nc.tensor.matmul(q[:, :], wt[:, 0, ko, :],
                                                 xs1, start=st,
                                                 stop=sp and l0mm == 3)
                            for p, wt in zip(ps, wts):
                                nc.tensor.matmul(p[:, :], wt[:, 1, ko, :],
                                                 xs0, start=False, stop=sp)
                            if l0mm == 4:
                                for q, wt in zip(qs, wts):
                                    nc.tensor.matmul(q[:, :],
                                                     wt[:, 1, ko, :], xs1,
                                                     start=False, stop=sp)
                        for j in range(g):
                            t2 = tp.tile([128, fc], mybir.dt.float32,
                                         tag="t2", bufs=4)
                            nc.scalar.mul(t2[:, :], qs[j][:, :], S1)
                            nc.vector.scalar_tensor_tensor(
                                cur[:, mo0 + j, :fc], t2[:, :], 1.0,
                                ps[j][:, :],
                                op0=mybir.AluOpType.mult,
                                op1=mybir.AluOpType.add)
                    else:
                        rhs = cur_in
                        for ko in range(KO):
                            for i in range(nterms):
                                for p, wt in zip(ps, wts):
                                    nc.tensor.matmul(
                                        p[:, :], wt[:, i, ko, :],
                                        rhs[:, ko, f0:f0 + fc],
                                        start=(ko == 0 and i == 0),
                                        stop=(ko == KO - 1
                                              and i == nterms - 1))
                        for j in range(g):
                            # On the kernel's final chunk, alternate the
                            # PSUM->SBUF copies between ACT and DVE so the
                            # last scan isn't serialized behind one engine.
                            if dve_copy and (mo0 // g + j) % 2 == 1:
                                nc.vector.tensor_copy(cur[:, mo0 + j, :fc],
                                                      ps[j][:, :])
                            else:
                                nc.scalar.copy(cur[:, mo0 + j, :fc],
                                               ps[j][:, :])

                def scan(l, spk, vts, cur, f0, tcs, ts0):
                    vt = vts[0]
                    for ts in range(tcs):
                        tl = slice(ts * BL, (ts + 1) * BL)
                        gl = slice(f0 + ts * BL, f0 + (ts + 1) * BL)
                        nc.vector.scalar_tensor_tensor(
                            vt[:, :, :], vt[:, :, :], ALPHA, cur[:, :, tl],
                            op0=mybir.AluOpType.mult, op1=mybir.AluOpType.add)
                        nc.vector.tensor_scalar(
                            spk[:, :, gl], vt[:, :, :], THRESH, None,
                            op0=mybir.AluOpType.is_ge)
                        nc.vector.scalar_tensor_tensor(
                            vt[:, :, :], vt[:, :, :], THRESH, vt[:, :, :],
                            op0=mybir.AluOpType.is_lt,
                            op1=mybir.AluOpType.mult)

                cur_in = None
                for l in range(NL):
                    last = l == NL - 1
                    spk = None if last else actsp.tile([128, NO, F], f16,
                                                       tag="spk", bufs=2)
                    vts = [vp.tile([128, NO, BL], mybir.dt.float32,
                                   tag=f"v{j}", bufs=1, name=f"v{j}_{l}")
                           for j in range(1)]
                    nc.vector.memset(vts[0][:, :, :], 0.0)
                    # Resident weight tiles: one DMA per layer for mo < RES,
                    # issued lazily in chunk-0 GEMM order so streamed tiles
                    # don't queue behind the whole resident set in the SP
                    # FIFO; per-mo tags let the next layer's load overlap
                    # this layer's tail (WAR resolves per-tile).
                    wres = [None] * RES
                    tsp_l = layer_ts[l]
                    fsplit = [t * BL for t in tsp_l]
                    foffs = [sum(fsplit[:i]) for i in range(len(fsplit))]
                    for c, (f0, fc, tcs) in enumerate(
                            zip(foffs, fsplit, tsp_l)):
                        cur = curp.tile([128, NO, fmax],
                                        mybir.dt.float32, tag="cur",
                                        name=f"cur_{l}_{c}")
                        gbase = 2 if l == 0 else GRP
                        groups = []
                        m = 0
                        while m < NO:
                            g = gbase if m + gbase <= RES else 2
                            groups.append((m, g))
                            m += g
                        for mo0, g in groups:
                            wts = []
                            for mo in range(mo0, mo0 + g):
                                if mo < RES:
                                    if wres[mo] is None:
                                        wt = wp.tile(
                                            [128, nterms, KO, 128], f16,
                                            tag=f"wr{mo}", bufs=1,
                                            name=f"wr{mo}_{l}")
                                        nc.sync.dma_start(wt[:, :, :, :],
                                                          wh_d[l, mo])
                                        wres[mo] = wt
                                    wts.append(wres[mo])
                                else:
                                    wt = wp.tile([128, nterms, KO, 128],
                                                 f16, tag="ws", bufs=WSRING)
                                    nc.sync.dma_start(wt[:, :, :, :],
                                                      wh_d[l, mo])
                                    wts.append(wt)
                            gemm_grp(l, mo0, wts, cur, cur_in, f0, fc,
                                     dve_copy=(last and
                                               c == len(tsp_l) - 1))
                        if last:
                            spk = actsp.tile([128, NO, fmax],
                                             mybir.dt.float32, tag="o32",
                                             bufs=2)
                            scan(l, spk, vts, cur, 0, tcs, f0 // BL)
                            # Issue from the otherwise-idle gpsimd queue so
                            # the wait-for-scan doesn't head-of-line block the
                            # ACT (PSUM copies) or SP (weight stream) rings.
                            nc.gpsimd.dma_start(out_d[:, :, f0:f0 + fc],
                                                spk[:, :, :fc])
                        else:
                            scan(l, spk, vts, cur, f0, tcs, f0 // BL)
                    cur_in = spk

            if reps == 1:
                body()
            else:
                with tctx.For_i(0, reps, 1) as iv:
                    body(iv)
    nc.compile()
    return nc


def _chunk(wm):
    """Wm [m, n] fp32 -> [mo, p, ko, mi] contiguous with lhsT layout."""
    wmT = np.ascontiguousarray(wm.T)
    return np.ascontiguousarray(
        wmT.reshape(KO, 128, NO, 128).transpose(2, 1, 0, 3))


def prep_weights(inputs, nterms=NT):
    """Returns dict of weight arrays for in_maps: fp16 term-split, 2^12-scaled."""
    wh = np.empty((NL, NO, 128, nterms, KO, 128), np.float16)
    for l in range(NL):
        wm = (np.asarray(inputs[f"W{l}"], np.float32)
              * np.asarray(inputs[f"mask{l}"]).astype(np.float32))
        wc = _chunk(wm).astype(np.float32) * np.float32(WSCALE)
        h0 = wc.astype(np.float16)
        r1 = wc - h0.astype(np.float32)
        # h1 carries its true (final) scale so both terms share a PSUM bank.
        h1 = r1.astype(np.float16)
        wh[l, :, :, 0] = h0
        wh[l, :, :, 1] = h1
        if nterms == 3:
            r2 = r1 - h1.astype(np.float32)
            wh[l, :, :, 2] = r2.astype(np.float16)
    return {"wh": wh}


def prep_x(x_core):
    """x_core [BL, T, N] -> [2, 128, KO, F] fp16 split, f = t*BL+b."""
    xt = x_core.transpose(2, 1, 0)                 # [n, t, b]
    xt = xt.reshape(KO, 128, T, BL).transpose(1, 0, 2, 3)  # [p, ko, t, b]
    xt = np.ascontiguousarray(xt.reshape(128, KO, F), dtype=np.float32)
    x0 = xt.astype(np.float16)
    x1 = ((xt - x0.astype(np.float32))
          * np.float32(2.0 ** 11)).astype(np.float16)
    return np.stack([x0, x1])


def unprep_out(o):
    """[128, NO, F] -> [BL, T, N]."""
    o = o.reshape(128, NO, T, BL).transpose(1, 0, 2, 3)    # [no, p, t, b]
    o = o.reshape(N, T, BL).transpose(2, 1, 0)             # [b, t, n]
    return np.ascontiguousarray(o)


_cached_nc = None


def kernel(**inputs) -> np.ndarray:
    global _cached_nc
    from concourse.bass_utils import run_bass_kernel_spmd

    if _cached_nc is None:
        _cached_nc = build(reps=1)
    nc = _cached_nc

    wmaps = prep_weights(inputs)
    x = np.asarray(inputs["x"], np.float32)
    in_maps = [dict(wmaps, x=prep_x(x[ci * BL:(ci + 1) * BL]))
               for ci in range(NCORES)]
    res = run_bass_kernel_spmd(nc, in_maps, core_ids=list(range(NCORES)))
    out = np.empty((B, T, N), np.float32)
    for ci in range(NCORES):
        out[ci * BL:(ci + 1) * BL] = unprep_out(res.results[ci]["out"])
    return out
